# revision 5
# baseline (speedup 1.0000x reference)
"""ChebConv (K=5) Trainium2 Bass kernel, 8-core SPMD.

Math (per reference): x0 = x transposed to [V, D]; T_0=x0, T_1=L@x0,
T_k = 2L@T_{k-1} - T_{k-2}; out[b,fout,v,xyz] = sum_{k,fin} T_k[v,(fin,b,xyz)]
* W[k,fin,fout] + bias.

Strategy:
- Shard D = Fin*B*X*Y*Z over 8 cores by (b, x-pair): core i handles
  b = i//4, x in {2*(i%4), 2*(i%4)+1}  ->  per-core D_loc = 32fin * 128xyz.
  Per-core x0 slice layout: [V=768, 4096] with column d = xz*32 + fin.
- The sparse L (768x768, ~6k nnz) is densified on the host; each spmm is a
  dense [768,768] @ [768, chunk] matmul on TensorE in float32r (full-rate
  fp32 mode). Chebyshev recurrence runs on PSUM->SBUF copies with the
  axpy fused into the copy (DVE).
- The final (K*Fin x Fout) GEMM contracts fin, which lives in the free
  axis -> each basis chunk is cast to bf16 (ACT) and PE-transposed to
  [(xz4,fin32), vo] layout, then a block-diagonal W (bf16) accumulates all
  K,fin into PSUM with vo back on partitions.
- Output per core is [V=768, xz*Fout=4096] f32, reassembled on host.
"""

import numpy as np
import ml_dtypes

B, FIN, V, X, Y, Z = 2, 32, 768, 8, 8, 8
K, FOUT = 5, 32
XYZ = X * Y * Z
NCORES = 8
XZL = 128            # xyz positions per core (2 x-planes * 64)
DLOC = XZL * FIN     # 4096 columns per core
VT = V // 128        # 6 v partition tiles
CH = 512             # chunk columns (16 xz * 32 fin)
NCH = DLOC // CH     # 8 chunks
XZC = CH // FIN      # 16 xz per chunk
DB = CH // 128       # 4 d-blocks (of 128) per chunk

_cache = {}


def _build_nc(reps=1, stages=("spmm", "tr", "gemm")):
    import concourse.bass as bass
    import concourse.bacc as bacc
    import concourse.mybir as mybir
    from concourse.tile import TileContext
    import contextlib

    f32 = mybir.dt.float32
    f32r = mybir.dt.float32r
    bf16 = mybir.dt.bfloat16

    nc = bacc.Bacc(None, target_bir_lowering=False)
    xs = nc.declare_dram_parameter("xs", [V, DLOC], f32r, isOutput=False)
    lt2b = nc.declare_dram_parameter("lt2b", [128, VT * VT * 128], f32r, isOutput=False)
    wtb = nc.declare_dram_parameter("wtb", [128, K * 128], bf16, isOutput=False)
    ident = nc.declare_dram_parameter("ident", [128, 128], bf16, isOutput=False)
    biasr = nc.declare_dram_parameter("biasr", [128, CH], f32, isOutput=False)
    outp = nc.declare_dram_parameter("outp", [V, XZL * FOUT], f32, isOutput=True)

    with TileContext(nc) as tc:
        with (
            tc.tile_pool(name="consts", bufs=1) as cpool,
            tc.tile_pool(name="xgen", bufs=1) as xpool,
            tc.tile_pool(name="x0load", bufs=2) as lpool,
            tc.tile_pool(name="xcast", bufs=2) as bpool,
            tc.tile_pool(name="basisT", bufs=2) as tpool,
            tc.tile_pool(name="osb", bufs=2) as opool,
            tc.tile_pool(name="zp", bufs=3, space="PSUM") as zpool,
            tc.tile_pool(name="ptp", bufs=2, space="PSUM") as ptpool,
            tc.tile_pool(name="pop", bufs=2, space="PSUM") as popool,
        ):
            lt2_sb = cpool.tile([128, VT * VT * 128], f32r)
            wt_sb = cpool.tile([128, K * 128], bf16)
            id_sb = cpool.tile([128, 128], bf16)
            bias_sb = cpool.tile([128, CH], f32)
            nc.sync.dma_start(out=lt2_sb[:], in_=lt2b[:])
            nc.sync.dma_start(out=wt_sb[:], in_=wtb[:])
            nc.sync.dma_start(out=id_sb[:], in_=ident[:])
            nc.sync.dma_start(out=bias_sb[:], in_=biasr[:])

            def lt2_blk(vi, vo):
                s = (vi * VT + vo) * 128
                return lt2_sb[:, s:s + 128]

            if reps > 1:
                rep_cm = tc.For_i(
                    0, reps, 1,
                    hint_engines=(mybir.EngineType.PE, mybir.EngineType.DVE,
                                  mybir.EngineType.Activation,
                                  mybir.EngineType.SP))
            else:
                rep_cm = contextlib.nullcontext()
            with rep_cm:
              for c in range(NCH):
                # ---- load x0 chunk ----
                x0 = []
                for vt in range(VT):
                    t = lpool.tile([128, CH], f32r, tag=f"x0_{vt}")
                    nc.sync.dma_start(
                        out=t[:], in_=xs[vt * 128:(vt + 1) * 128, c * CH:(c + 1) * CH])
                    x0.append(t)

                # basis in normal orientation per k; basisT bf16 tiles per k
                basisT = {}

                def cast_and_transpose(k, xk):
                    # cast to bf16 (ACT), then PE-transpose into
                    # [(d%128) part, vo] bf16 tiles, one per d-block.
                    xb = []
                    for vt in range(VT):
                        tb = bpool.tile([128, CH], bf16, tag=f"xb_{vt}")
                        nc.scalar.copy(out=tb[:], in_=xk[vt][:].bitcast(f32))
                        xb.append(tb)
                    tiles = []
                    for j in range(DB):
                        pt = ptpool.tile([128, V], bf16, tag="pt")
                        for vt in range(VT):
                            nc.tensor.transpose(
                                pt[:, vt * 128:(vt + 1) * 128],
                                xb[vt][:, j * 128:(j + 1) * 128],
                                id_sb[:],
                            )
                        st = tpool.tile([128, V], bf16, tag=f"bT_{k}_{j}")
                        nc.vector.tensor_copy(st[:], pt[:])
                        tiles.append(st)
                    basisT[k] = tiles

                cast_and_transpose(0, x0)

                xprev2, xprev1 = None, x0
                for k in range(1, K):
                    xk = []
                    for vt in range(VT):
                        z = zpool.tile([128, CH], f32, tag="z")
                        for vi in range(VT):
                            nc.tensor.matmul(
                                z[:], lt2_blk(vi, vt),
                                xprev1[vi][:],
                                start=(vi == 0), stop=(vi == VT - 1),
                            )
                        t = xpool.tile([128, CH], f32r, tag=f"x{k}_{vt}")
                        if k == 1:
                            # z = 2L x0 ; T_1 = L x0 = z/2
                            nc.vector.tensor_scalar_mul(t[:], z[:], 0.5)
                        else:
                            # T_k = 2L T_{k-1} - T_{k-2}
                            nc.vector.tensor_sub(t[:], z[:], xprev2[vt][:].bitcast(f32))
                        xk.append(t)
                    cast_and_transpose(k, xk)
                    xprev2, xprev1 = xprev1, xk

                # ---- final GEMM: out[vo, (xz,fo)] over (k, fin) ----
                for vt in range(VT):
                    po = popool.tile([128, CH], f32, tag="po")
                    for j in range(DB):
                        for k in range(K):
                            nc.tensor.matmul(
                                po[:, j * 128:(j + 1) * 128],
                                basisT[k][j][:, vt * 128:(vt + 1) * 128],
                                wt_sb[:, k * 128:(k + 1) * 128],
                                start=(k == 0), stop=(k == K - 1),
                            )
                    ot = opool.tile([128, CH], f32, tag=f"o_{vt}")
                    nc.vector.tensor_add(ot[:], po[:], bias_sb[:])
                    nc.sync.dma_start(
                        out=outp[vt * 128:(vt + 1) * 128, c * CH:(c + 1) * CH],
                        in_=ot[:])
    nc.compile()
    return nc


def _host_prep(lap_rows, lap_cols, lap_vals, x, weight, bias):
    """Build per-core input maps."""
    L = np.zeros((V, V), np.float32)
    np.add.at(L, (np.asarray(lap_rows), np.asarray(lap_cols)),
              np.asarray(lap_vals, np.float32))
    LT2 = (2.0 * L).T.astype(np.float32)
    lt2b = np.ascontiguousarray(
        LT2.reshape(VT, 128, VT, 128).transpose(1, 0, 2, 3).reshape(128, -1))

    w = np.asarray(weight, np.float32)
    wt = np.zeros((K, 128, 128), np.float32)
    for xz in range(4):
        wt[:, xz * FIN:(xz + 1) * FIN, xz * FOUT:(xz + 1) * FOUT] = w
    wtb = np.ascontiguousarray(
        wt.transpose(1, 0, 2).reshape(128, K * 128)).astype(ml_dtypes.bfloat16)

    identity = np.eye(128, dtype=ml_dtypes.bfloat16)
    biasr = np.tile(np.asarray(bias, np.float32), (128, XZC)).astype(np.float32)
    assert biasr.shape == (128, CH)

    xf = np.asarray(x, np.float32)
    in_maps = []
    for i in range(NCORES):
        b, xp = i // 4, i % 4
        xsl = xf[b][:, :, 2 * xp:2 * xp + 2]          # [FIN, V, 2, Y, Z]
        xs = np.ascontiguousarray(
            xsl.transpose(1, 2, 3, 4, 0).reshape(V, DLOC))
        in_maps.append({
            "xs": xs, "lt2b": lt2b, "wtb": wtb,
            "ident": identity, "biasr": biasr,
        })
    return in_maps


def kernel(lap_rows, lap_cols, lap_vals, x, weight, bias):
    from concourse.bass_utils import run_bass_kernel_spmd

    if "nc" not in _cache:
        _cache["nc"] = _build_nc()
    nc = _cache["nc"]

    in_maps = _host_prep(lap_rows, lap_cols, lap_vals, x, weight, bias)
    res = run_bass_kernel_spmd(nc, in_maps, core_ids=list(range(NCORES)))

    out = np.empty((B, FOUT, V, X, Y, Z), np.float32)
    for i in range(NCORES):
        b, xp = i // 4, i % 4
        o = res.results[i]["outp"].reshape(V, 2, Y, Z, FOUT)
        out[b, :, :, 2 * xp:2 * xp + 2] = o.transpose(4, 0, 1, 2, 3)
    return out
